# revision 25
# baseline (speedup 1.0000x reference)
"""Trainium2 Bass kernel for EquivariantBallUnpooling.

Math (per parent node n):
  out[n, 0:576] = x_comb[n, 0:577] @ W_combined
where x_comb = [x_mv flattened (i,k), x_s, 1.0] and W_combined is built host-side
from the small EquiLinear weights and the 9-element Pin(3,0,1) basis. The basis
is block-sparse: output blade j receives only input blades {j, e0src(j)}.

Children = repeat_interleave(parents, 2), so all compute happens on parents; a
parent's result is added to both of its children's skip rows on-chip.

Blade tiling: the e0-wedge map pairs blades (0,1),(2,5),(3,6),(4,7),(8,11),
(9,12),(10,13),(14,15). x^T rows are permuted so each pair lives in one
128-row K-tile -> each K-tile feeds exactly its own 128 output columns:
4 dense [128x128] matmuls, no overlap, plus 3 small ones for the scalar head.

Per-core structure (pure data parallel over parents, 8 cores):
  per 128-parent block:
   - contiguous DMA loads (x 256KB, x_s 32KB, skip pair-tiles 512/64KB)
   - ACT permutes features to pair-order, PE transposes to x^T (PSUM->ACT->SBUF)
   - 8 matmuls accumulate out[128 parents, 576] in PSUM
   - ACT drains PSUM->SBUF undoing the pair permutation; DVE adds skip;
     contiguous stores (children of a parent share its partition)
"""

import numpy as np

_GRADES = np.array([0, 1, 1, 1, 1, 2, 2, 2, 2, 2, 2, 3, 3, 3, 3, 4])
_E0_MAP = {0: 1, 2: 5, 3: 6, 4: 7, 8: 11, 9: 12, 10: 13, 14: 15}

# (src, dst) wedge pairs, ordered so tiles T_t = pairs (2t, 2t+1)
_PAIRS = [(0, 1), (14, 15), (2, 5), (3, 6), (4, 7), (8, 11), (9, 12), (10, 13)]
_SIGMA = [j for p in _PAIRS for j in p]  # psum/xT position q -> blade

N_PARENT_FULL = 65536
N_CORES = 8
STRIDE = 2
NP = N_PARENT_FULL // N_CORES  # parents per core
BLK = 128

# float32r: fp32 with the mantissa truncated to 11 bits (TF32-like) but fp32
# PSUM accumulation; single-pass matmul (half the PE row-cost of fp32, one
# LDWEIGHTS instead of two). Input rounding error ~2.4e-4 relative.
FP32R_MM = True


def build_program(np_parents=NP):
    import concourse.mybir as mybir
    import concourse.tile as tile
    from concourse import bacc
    from concourse.masks import make_identity

    f32 = mybir.dt.float32
    xdt = mybir.dt.float32r if FP32R_MM else f32
    nch = np_parents * STRIDE
    nblk = np_parents // BLK
    assert np_parents % BLK == 0

    nc = bacc.Bacc("TRN2", target_bir_lowering=False, debug=False)

    x_mv = nc.dram_tensor("x_mv", [np_parents, 512], f32, kind="ExternalInput").ap()
    x_s = nc.dram_tensor("x_s", [np_parents, 64], f32, kind="ExternalInput").ap()
    skip_mv = nc.dram_tensor("skip_mv", [nch, 512], f32, kind="ExternalInput").ap()
    skip_s = nc.dram_tensor("skip_s", [nch, 64], f32, kind="ExternalInput").ap()

    # single packed weight tensor -> one DMA, one wait source for the matmuls
    wcat = nc.dram_tensor("wcat", [128, 864], f32, kind="ExternalInput").ap()

    out_mv = nc.dram_tensor("out_mv", [nch, 512], f32, kind="ExternalOutput").ap()
    out_s = nc.dram_tensor("out_s", [nch, 64], f32, kind="ExternalOutput").ap()

    with tile.TileContext(nc) as tc:
        with (
            tc.tile_pool(name="const", bufs=1) as cpool,
            tc.tile_pool(name="io", bufs=4) as iopool,
            tc.tile_pool(name="xt", bufs=3) as xtpool,
            tc.tile_pool(name="ps", bufs=2, space="PSUM") as pspool,
        ):
            ident = cpool.tile([128, 128], f32)
            make_identity(nc, ident[:])
            ones = cpool.tile([1, 128], f32)
            nc.gpsimd.memset(ones[:], 1.0)
            wt = cpool.tile([128, 864], f32)
            nc.sync.dma_start(wt[:], wcat)
            # rounded copies for the fp32r matmul path (ACT rounds on copy)
            wtr = cpool.tile([128, 864], xdt)
            nc.scalar.copy(wtr[:], wt[:])
            onesr = cpool.tile([1, 128], xdt)
            nc.scalar.copy(onesr[:], ones[:])

            # PE matmuls tolerate few semaphore waits; funnel PE deps through
            # ACT. These dummies absorb the one-time gpsimd (identity/ones)
            # and weight-DMA waits into PE's observed vector clock; the ACT
            # reads keep the psum slot releases on the ACT semaphore.
            d1 = pspool.tile([128, 512], f32, tag="ptA")
            nc.tensor.transpose(d1[:, 0:128], ident[:], ident[:])
            d2 = pspool.tile([128, 512], f32, tag="ptA")
            nc.tensor.transpose(d2[:, 0:128], wt[:, 0:128], ident[:])
            scratch = cpool.tile([128, 128], f32)
            nc.scalar.copy(scratch[:], d1[:, 0:128])
            nc.scalar.copy(scratch[:], d2[:, 0:128])

            for b in range(nblk):
                p0 = b * BLK
                c0 = p0 * STRIDE
                # --- load x block ---
                xc = iopool.tile([BLK, 512], f32, tag="xc")
                nc.sync.dma_start(xc[:], x_mv[p0 : p0 + BLK, :])
                xcs = iopool.tile([BLK, 64], f32, tag="xcs")
                nc.sync.dma_start(xcs[:], x_s[p0 : p0 + BLK, :])

                # --- ACT: permute mv features (i,k) -> pair-ordered (q,i) so
                # x^T comes out with wedge pairs co-located per K-tile ---
                xcp = iopool.tile([BLK, 576], f32, tag="xcp")
                x3 = xc[:].rearrange("p (i k) -> p k i", k=16)  # [p, 16, 32]
                xcp3 = xcp[:, 0:512].rearrange("p (q i) -> p q i", i=32)
                for q, (s, e) in enumerate(_PAIRS):
                    nc.scalar.copy(
                        xcp3[:, 2 * q : 2 * q + 2, :], x3[:, s : e + 1 : e - s, :]
                    )
                nc.scalar.copy(xcp[:, 512:576], xcs[:])

                ptA = pspool.tile([128, 512], f32, tag="ptA")
                for g in range(4):
                    nc.tensor.transpose(
                        ptA[:, g * 128 : (g + 1) * 128],
                        xcp[:, g * 128 : (g + 1) * 128],
                        ident[:],
                    )
                ptB = pspool.tile([64, 128], f32, tag="ptB")
                nc.tensor.transpose(ptB[:], xcp[:, 512:576], ident[:])

                xt = xtpool.tile([128, 512], xdt, tag="xt")
                nc.scalar.copy(xt[:], ptA[:])
                xts = xtpool.tile([64, 128], xdt, tag="xts")
                nc.scalar.copy(xts[:], ptB[:])

                # --- matmuls (psum col = q*32+o; each K-tile -> its own cols) ---
                poA = pspool.tile([128, 512], f32, tag="poA")
                poB = pspool.tile([128, 64], f32, tag="poB")
                mm = nc.tensor.matmul
                for t in range(4):
                    cs = slice(128 * t, 128 * (t + 1))
                    mm(poA[:, cs], xt[:, cs], wtr[:, cs], start=(t == 0), stop=False)
                mm(poA[:, 0:32], xts[:], wtr[0:64, 704:736], start=False, stop=True)
                mm(poB[:], xt[:, 0:128], wtr[:, 640:704], start=True, stop=False)
                mm(poB[:], xts[:], wtr[0:64, 736:800], start=False, stop=False)
                mm(poB[:], onesr[:], wtr[0:1, 800:864], start=False, stop=True)

                # --- ACT drains PSUM -> SBUF, undoing the pair permutation ---
                oA = xtpool.tile([BLK, 512], f32, tag="oA")
                oA3 = oA[:].rearrange("p (o j) -> p o j", j=16)
                poAq = poA[:].rearrange("p (q o) -> p q o", o=32)
                for q, (s, e) in enumerate(_PAIRS):
                    nc.scalar.copy(
                        oA3[:, :, s : e + 1 : e - s],
                        poAq[:, 2 * q : 2 * q + 2, :].transpose([0, 2, 1]),
                    )
                oB = xtpool.tile([BLK, 64], f32, tag="oB")
                nc.scalar.copy(oB[:], poB[:])

                # --- skip add + store ---
                skm = iopool.tile([BLK, 1024], f32, tag="skm")
                nc.sync.dma_start(
                    skm[:],
                    skip_mv[c0 : c0 + 2 * BLK, :].rearrange("(p a) f -> p (a f)", a=2),
                )
                sks = iopool.tile([BLK, 128], f32, tag="sks")
                nc.sync.dma_start(
                    sks[:],
                    skip_s[c0 : c0 + 2 * BLK, :].rearrange("(p a) f -> p (a f)", a=2),
                )
                om = iopool.tile([BLK, 1024], f32, tag="om")
                os_ = iopool.tile([BLK, 128], f32, tag="os")
                nc.vector.tensor_add(om[:, 0:512], oA[:], skm[:, 0:512])
                nc.vector.tensor_add(om[:, 512:1024], oA[:], skm[:, 512:1024])
                nc.vector.tensor_add(os_[:, 0:64], oB[:], sks[:, 0:64])
                nc.vector.tensor_add(os_[:, 64:128], oB[:], sks[:, 64:128])
                nc.scalar.dma_start(
                    out_mv[c0 : c0 + 2 * BLK, :].rearrange("(p a) f -> p (a f)", a=2),
                    om[:],
                )
                nc.scalar.dma_start(
                    out_s[c0 : c0 + 2 * BLK, :].rearrange("(p a) f -> p (a f)", a=2),
                    os_[:],
                )
    nc.compile()
    return nc


def pack_weights(w_mv, w_s2mv, w_mv2s, w_s2s, b_s):
    """Build the combined [577, 576] weight, then permute rows/cols into the
    pair-ordered layout and concatenate all matmul blocks into one tensor."""
    w_mv = np.asarray(w_mv, np.float32)
    W = np.zeros((577, 576), np.float32)
    for j in range(16):
        W[j * 32 : (j + 1) * 32, j:512:16] = w_mv[:, :, _GRADES[j]].T
    for src, dst in _E0_MAP.items():
        W[src * 32 : (src + 1) * 32, dst:512:16] += w_mv[:, :, 5 + _GRADES[src]].T
    W[512:576, 0:512:16] = np.asarray(w_s2mv, np.float32).T
    W[0:32, 512:576] = np.asarray(w_mv2s, np.float32).T
    W[512:576, 512:576] += np.asarray(w_s2s, np.float32).T
    W[576, 512:576] = np.asarray(b_s, np.float32)

    # permute rows and mv-cols into pair order: position q <-> blade SIGMA[q]
    row_perm = [_SIGMA[q] * 32 + i for q in range(16) for i in range(32)]
    col_perm = [o * 16 + _SIGMA[q] for q in range(16) for o in range(32)]
    Wmv = W[:, :512][row_perm + list(range(512, 577))][:, col_perm]
    Ws = W[:, 512:576]

    wcat = np.zeros((128, 864), np.float32)
    for t in range(4):
        wcat[:, 128 * t : 128 * (t + 1)] = Wmv[128 * t : 128 * (t + 1),
                                               128 * t : 128 * (t + 1)]
    wcat[:, 640:704] = Ws[row_perm[0:128]]        # out_s from tile-0 rows
    wcat[0:64, 704:736] = Wmv[512:576, 0:32]      # s2mv -> blade 0 (position 0)
    wcat[0:64, 736:800] = Ws[512:576]             # s2s
    wcat[0, 800:864] = Ws[576]                    # bias via ones row
    return {"wcat": wcat}


_cached_nc = None
TRACE = False  # set True (e.g. from test.py) to capture an NTFF profile
LAST_RESULTS = None  # BassKernelResults of the most recent kernel() call


def kernel(x_mv, x_s, skip_mv, skip_s, w_mv, w_s2mv, w_mv2s, w_s2s, b_s, stride):
    global _cached_nc, LAST_RESULTS
    from concourse.bass_utils import run_bass_kernel_spmd

    assert int(stride) == STRIDE
    x_mv = np.ascontiguousarray(np.asarray(x_mv, np.float32).reshape(N_PARENT_FULL, 512))
    x_s = np.ascontiguousarray(np.asarray(x_s, np.float32))
    n_child = N_PARENT_FULL * STRIDE
    skip_mv_f = np.ascontiguousarray(np.asarray(skip_mv, np.float32).reshape(n_child, 512))
    skip_s = np.ascontiguousarray(np.asarray(skip_s, np.float32))

    packed = pack_weights(w_mv, w_s2mv, w_mv2s, w_s2s, b_s)

    if _cached_nc is None:
        _cached_nc = build_program(NP)
    nc = _cached_nc

    nch = NP * STRIDE
    in_maps = []
    for c in range(N_CORES):
        m = {
            "x_mv": x_mv[c * NP : (c + 1) * NP],
            "x_s": x_s[c * NP : (c + 1) * NP],
            "skip_mv": skip_mv_f[c * nch : (c + 1) * nch],
            "skip_s": skip_s[c * nch : (c + 1) * nch],
        }
        m.update(packed)
        in_maps.append(m)

    res = run_bass_kernel_spmd(nc, in_maps, core_ids=list(range(N_CORES)), trace=TRACE)
    LAST_RESULTS = res
    out_mv = np.concatenate([r["out_mv"] for r in res.results], axis=0)
    out_s = np.concatenate([r["out_s"] for r in res.results], axis=0)
    return out_mv.reshape(n_child, 32, 16), out_s


# revision 29
# speedup vs baseline: 1.5249x; 1.5249x over previous
"""Trainium2 Bass kernel for EquivariantBallUnpooling.

Math (per parent node n):
  out[n, 0:576] = x_comb[n, 0:577] @ W_combined
where x_comb = [x_mv flattened (i,k), x_s, 1.0] and W_combined is built host-side
from the small EquiLinear weights and the 9-element Pin(3,0,1) basis. The basis
is block-sparse: output blade j receives only input blades {j, e0src(j)}.

Children = repeat_interleave(parents, 2), so all compute happens on parents; a
parent's result is added to both of its children's skip rows on-chip.

Blade tiling: the e0-wedge map pairs blades (0,1),(2,5),(3,6),(4,7),(8,11),
(9,12),(10,13),(14,15). x^T rows are permuted so each pair lives in one
128-row K-tile -> each K-tile feeds exactly its own 128 output columns:
4 dense [128x128] matmuls, no overlap, plus 3 small ones for the scalar head.

Per-core structure (pure data parallel over parents, 8 cores):
  per 128-parent block:
   - contiguous DMA loads (x 256KB, x_s 32KB, skip pair-tiles 512/64KB)
   - ACT permutes features to pair-order, PE transposes to x^T (PSUM->ACT->SBUF)
   - 8 matmuls accumulate out[128 parents, 576] in PSUM
   - ACT drains PSUM->SBUF undoing the pair permutation; DVE adds skip;
     contiguous stores (children of a parent share its partition)
"""

import numpy as np

_GRADES = np.array([0, 1, 1, 1, 1, 2, 2, 2, 2, 2, 2, 3, 3, 3, 3, 4])
_E0_MAP = {0: 1, 2: 5, 3: 6, 4: 7, 8: 11, 9: 12, 10: 13, 14: 15}

# x^T position q -> blade. Tiles (4 positions each) co-locate the e0-wedge
# pairs: T0={2,5,8,11} T1={3,6,9,12} T2={4,7,10,13} T3={0,1,14,15}. Positions
# 0..11 form the uniform lattice blade = 2 + t + 3b (q = 4t + b), so the
# permute/unpermute copies for 3 of 4 tiles collapse into one rearrange.
_SIGMA = [2, 5, 8, 11, 3, 6, 9, 12, 4, 7, 10, 13, 0, 1, 14, 15]

N_PARENT_FULL = 65536
N_CORES = 8
STRIDE = 2
NP = N_PARENT_FULL // N_CORES  # parents per core
BLK = 128

# float32r: fp32 with the mantissa truncated to 11 bits (TF32-like) but fp32
# PSUM accumulation; single-pass matmul (half the PE row-cost of fp32, one
# LDWEIGHTS instead of two). Input rounding error ~2.4e-4 relative.
FP32R_MM = True


def build_program(np_parents=NP):
    import concourse.mybir as mybir
    import concourse.tile as tile
    from concourse import bacc
    from concourse.masks import make_identity

    f32 = mybir.dt.float32
    xdt = mybir.dt.float32r if FP32R_MM else f32
    nch = np_parents * STRIDE
    nblk = np_parents // BLK
    assert np_parents % BLK == 0

    nc = bacc.Bacc("TRN2", target_bir_lowering=False, debug=False)

    x_mv = nc.dram_tensor("x_mv", [np_parents, 512], f32, kind="ExternalInput").ap()
    x_s = nc.dram_tensor("x_s", [np_parents, 64], f32, kind="ExternalInput").ap()
    skip_mv = nc.dram_tensor("skip_mv", [nch, 512], f32, kind="ExternalInput").ap()
    skip_s = nc.dram_tensor("skip_s", [nch, 64], f32, kind="ExternalInput").ap()

    # single packed weight tensor -> one DMA, one wait source for the matmuls
    wcat = nc.dram_tensor("wcat", [128, 864], f32, kind="ExternalInput").ap()

    out_mv = nc.dram_tensor("out_mv", [nch, 512], f32, kind="ExternalOutput").ap()
    out_s = nc.dram_tensor("out_s", [nch, 64], f32, kind="ExternalOutput").ap()

    with tile.TileContext(nc) as tc:
        with (
            tc.tile_pool(name="const", bufs=1) as cpool,
            tc.tile_pool(name="io", bufs=4) as iopool,
            tc.tile_pool(name="xt", bufs=3) as xtpool,
            tc.tile_pool(name="ps", bufs=2, space="PSUM") as pspool,
        ):
            ident = cpool.tile([128, 128], f32)
            make_identity(nc, ident[:])
            ones = cpool.tile([1, 128], f32)
            nc.gpsimd.memset(ones[:], 1.0)
            wt = cpool.tile([128, 864], f32)
            nc.sync.dma_start(wt[:], wcat)
            # rounded copies for the fp32r matmul path (ACT rounds on copy)
            wtr = cpool.tile([128, 864], xdt)
            nc.scalar.copy(wtr[:], wt[:])
            onesr = cpool.tile([1, 128], xdt)
            nc.scalar.copy(onesr[:], ones[:])

            # PE matmuls tolerate few semaphore waits; funnel PE deps through
            # ACT. These dummies absorb the one-time gpsimd (identity/ones)
            # and weight-DMA waits into PE's observed vector clock; the ACT
            # reads keep the psum slot releases on the ACT semaphore.
            d1 = pspool.tile([128, 512], f32, tag="ptA")
            nc.tensor.transpose(d1[:, 0:128], ident[:], ident[:])
            d2 = pspool.tile([128, 512], f32, tag="ptA")
            nc.tensor.transpose(d2[:, 0:128], wt[:, 0:128], ident[:])
            scratch = cpool.tile([128, 128], f32)
            nc.scalar.copy(scratch[:], d1[:, 0:128])
            nc.scalar.copy(scratch[:], d2[:, 0:128])

            for b in range(nblk):
                p0 = b * BLK
                c0 = p0 * STRIDE
                # --- load x block ---
                xc = iopool.tile([BLK, 512], f32, tag="xc")
                nc.sync.dma_start(xc[:], x_mv[p0 : p0 + BLK, :])
                xcs = iopool.tile([BLK, 64], f32, tag="xcs")
                nc.sync.dma_start(xcs[:], x_s[p0 : p0 + BLK, :])

                # --- ACT: permute mv features (i,k) -> pair-ordered (q,i) so
                # x^T comes out with wedge pairs co-located per K-tile ---
                xcp = iopool.tile([BLK, 512], f32, tag="xcp")
                x3 = xc[:].rearrange("p (i k) -> p k i", k=16)  # [p, 16, 32]
                xcp3 = xcp[:].rearrange("p (q i) -> p q i", i=32)
                # positions 0:12 <- blades 2+t+3b in one lattice copy
                nc.scalar.copy(
                    xcp3[:, 0:12, :].rearrange("p (t b) i -> p t b i", b=4),
                    x3[:, 2:14, :].rearrange("p (b t) i -> p t b i", t=3),
                )
                # tile 3: pairs (0,1) and (14,15)
                nc.scalar.copy(xcp3[:, 12:14, :], x3[:, 0:2, :])
                nc.scalar.copy(xcp3[:, 14:16, :], x3[:, 14:16, :])

                ptA = pspool.tile([128, 512], f32, tag="ptA")
                for g in range(4):
                    nc.tensor.transpose(
                        ptA[:, g * 128 : (g + 1) * 128],
                        xcp[:, g * 128 : (g + 1) * 128],
                        ident[:],
                    )
                ptB = pspool.tile([64, 128], f32, tag="ptB")
                nc.tensor.transpose(ptB[:], xcs[:], ident[:])

                xt = xtpool.tile([128, 512], xdt, tag="xt")
                nc.vector.tensor_copy(xt[:], ptA[:])
                xts = xtpool.tile([64, 128], xdt, tag="xts")
                nc.vector.tensor_copy(xts[:], ptB[:])

                # --- matmuls (psum col = q*32+o; each K-tile -> its own cols) ---
                poA = pspool.tile([128, 512], f32, tag="poA")
                poB = pspool.tile([128, 64], f32, tag="poB")
                mm = nc.tensor.matmul
                for t in range(4):
                    cs = slice(128 * t, 128 * (t + 1))
                    mm(poA[:, cs], xt[:, cs], wtr[:, cs], start=(t == 0), stop=False)
                # blade 0 sits at position 12 (tile 3)
                mm(poA[:, 384:416], xts[:], wtr[0:64, 704:736], start=False, stop=True)
                mm(poB[:], xt[:, 384:512], wtr[:, 640:704], start=True, stop=False)
                mm(poB[:], xts[:], wtr[0:64, 736:800], start=False, stop=False)
                mm(poB[:], onesr[:], wtr[0:1, 800:864], start=False, stop=True)

                # --- ACT drains PSUM -> SBUF, undoing the pair permutation ---
                oA = xtpool.tile([BLK, 512], f32, tag="oA")
                oA3 = oA[:].rearrange("p (o j) -> p o j", j=16)
                poAq = poA[:].rearrange("p (q o) -> p q o", o=32)
                nc.scalar.copy(
                    oA3[:, :, 2:14].rearrange("p o (b t) -> p o t b", t=3),
                    poA[:, 0:384].rearrange("p (t b o) -> p o t b", b=4, o=32),
                )
                nc.scalar.copy(
                    oA3[:, :, 0:2], poAq[:, 12:14, :].transpose([0, 2, 1])
                )
                nc.scalar.copy(
                    oA3[:, :, 14:16], poAq[:, 14:16, :].transpose([0, 2, 1])
                )

                # --- skip add + store (DVE reads poB straight from PSUM) ---
                skm = iopool.tile([BLK, 1024], f32, tag="skm")
                nc.gpsimd.dma_start(
                    skm[:],
                    skip_mv[c0 : c0 + 2 * BLK, :].rearrange("(p a) f -> p (a f)", a=2),
                )
                sks = iopool.tile([BLK, 128], f32, tag="sks")
                nc.gpsimd.dma_start(
                    sks[:],
                    skip_s[c0 : c0 + 2 * BLK, :].rearrange("(p a) f -> p (a f)", a=2),
                )
                om = iopool.tile([BLK, 1024], f32, tag="om")
                os_ = iopool.tile([BLK, 128], f32, tag="os")
                nc.vector.tensor_add(om[:, 0:512], oA[:], skm[:, 0:512])
                nc.vector.tensor_add(om[:, 512:1024], oA[:], skm[:, 512:1024])
                nc.vector.tensor_add(os_[:, 0:64], poB[:], sks[:, 0:64])
                nc.vector.tensor_add(os_[:, 64:128], poB[:], sks[:, 64:128])
                nc.scalar.dma_start(
                    out_mv[c0 : c0 + 2 * BLK, :].rearrange("(p a) f -> p (a f)", a=2),
                    om[:],
                )
                nc.scalar.dma_start(
                    out_s[c0 : c0 + 2 * BLK, :].rearrange("(p a) f -> p (a f)", a=2),
                    os_[:],
                )
    nc.compile()
    return nc


def pack_weights(w_mv, w_s2mv, w_mv2s, w_s2s, b_s):
    """Build the combined [577, 576] weight, then permute rows/cols into the
    pair-ordered layout and concatenate all matmul blocks into one tensor."""
    w_mv = np.asarray(w_mv, np.float32)
    W = np.zeros((577, 576), np.float32)
    for j in range(16):
        W[j * 32 : (j + 1) * 32, j:512:16] = w_mv[:, :, _GRADES[j]].T
    for src, dst in _E0_MAP.items():
        W[src * 32 : (src + 1) * 32, dst:512:16] += w_mv[:, :, 5 + _GRADES[src]].T
    W[512:576, 0:512:16] = np.asarray(w_s2mv, np.float32).T
    W[0:32, 512:576] = np.asarray(w_mv2s, np.float32).T
    W[512:576, 512:576] += np.asarray(w_s2s, np.float32).T
    W[576, 512:576] = np.asarray(b_s, np.float32)

    # permute rows and mv-cols into pair order: position q <-> blade SIGMA[q]
    row_perm = [_SIGMA[q] * 32 + i for q in range(16) for i in range(32)]
    col_perm = [o * 16 + _SIGMA[q] for q in range(16) for o in range(32)]
    Wmv = W[:, :512][row_perm + list(range(512, 577))][:, col_perm]
    Ws = W[:, 512:576]

    wcat = np.zeros((128, 864), np.float32)
    for t in range(4):
        wcat[:, 128 * t : 128 * (t + 1)] = Wmv[128 * t : 128 * (t + 1),
                                               128 * t : 128 * (t + 1)]
    wcat[:, 640:704] = Ws[row_perm[384:512]]      # out_s from tile-3 rows (blade 0)
    wcat[0:64, 704:736] = Wmv[512:576, 384:416]   # s2mv -> blade 0 (position 12)
    wcat[0:64, 736:800] = Ws[512:576]             # s2s
    wcat[0, 800:864] = Ws[576]                    # bias via ones row
    return {"wcat": wcat}


_cached_nc = None
TRACE = False  # set True (e.g. from test.py) to capture an NTFF profile
LAST_RESULTS = None  # BassKernelResults of the most recent kernel() call


def kernel(x_mv, x_s, skip_mv, skip_s, w_mv, w_s2mv, w_mv2s, w_s2s, b_s, stride):
    global _cached_nc, LAST_RESULTS
    from concourse.bass_utils import run_bass_kernel_spmd

    assert int(stride) == STRIDE
    x_mv = np.ascontiguousarray(np.asarray(x_mv, np.float32).reshape(N_PARENT_FULL, 512))
    x_s = np.ascontiguousarray(np.asarray(x_s, np.float32))
    n_child = N_PARENT_FULL * STRIDE
    skip_mv_f = np.ascontiguousarray(np.asarray(skip_mv, np.float32).reshape(n_child, 512))
    skip_s = np.ascontiguousarray(np.asarray(skip_s, np.float32))

    packed = pack_weights(w_mv, w_s2mv, w_mv2s, w_s2s, b_s)

    if _cached_nc is None:
        _cached_nc = build_program(NP)
    nc = _cached_nc

    nch = NP * STRIDE
    in_maps = []
    for c in range(N_CORES):
        m = {
            "x_mv": x_mv[c * NP : (c + 1) * NP],
            "x_s": x_s[c * NP : (c + 1) * NP],
            "skip_mv": skip_mv_f[c * nch : (c + 1) * nch],
            "skip_s": skip_s[c * nch : (c + 1) * nch],
        }
        m.update(packed)
        in_maps.append(m)

    res = run_bass_kernel_spmd(nc, in_maps, core_ids=list(range(N_CORES)), trace=TRACE)
    LAST_RESULTS = res
    out_mv = np.concatenate([r["out_mv"] for r in res.results], axis=0)
    out_s = np.concatenate([r["out_s"] for r in res.results], axis=0)
    return out_mv.reshape(n_child, 32, 16), out_s


# revision 31
# speedup vs baseline: 1.7018x; 1.1160x over previous
"""Trainium2 Bass kernel for EquivariantBallUnpooling.

Math (per parent node n):
  out[n, 0:576] = x_comb[n, 0:577] @ W_combined
where x_comb = [x_mv flattened (i,k), x_s, 1.0] and W_combined is built host-side
from the small EquiLinear weights and the 9-element Pin(3,0,1) basis. The basis
is block-sparse: output blade j receives only input blades {j, e0src(j)}.

Children = repeat_interleave(parents, 2), so all compute happens on parents; a
parent's result is added to both of its children's skip rows on-chip.

Blade tiling: the e0-wedge map pairs blades (0,1),(2,5),(3,6),(4,7),(8,11),
(9,12),(10,13),(14,15). x^T rows are permuted so each pair lives in one
128-row K-tile -> each K-tile feeds exactly its own 128 output columns:
4 dense [128x128] matmuls, no overlap, plus 3 small ones for the scalar head.

Per-core structure (pure data parallel over parents, 8 cores):
  per 128-parent block:
   - contiguous DMA loads (x 256KB, x_s 32KB, skip pair-tiles 512/64KB)
   - ACT permutes features to pair-order, PE transposes to x^T (PSUM->ACT->SBUF)
   - 8 matmuls accumulate out[128 parents, 576] in PSUM
   - ACT drains PSUM->SBUF undoing the pair permutation; DVE adds skip;
     contiguous stores (children of a parent share its partition)
"""

import numpy as np

_GRADES = np.array([0, 1, 1, 1, 1, 2, 2, 2, 2, 2, 2, 3, 3, 3, 3, 4])
_E0_MAP = {0: 1, 2: 5, 3: 6, 4: 7, 8: 11, 9: 12, 10: 13, 14: 15}

# x^T position q -> blade. Tiles (4 positions each) co-locate the e0-wedge
# pairs: T0={2,5,8,11} T1={3,6,9,12} T2={4,7,10,13} T3={0,1,14,15}. Positions
# 0..11 form the uniform lattice blade = 2 + t + 3b (q = 4t + b), so the
# permute/unpermute copies for 3 of 4 tiles collapse into one rearrange.
_SIGMA = [2, 5, 8, 11, 3, 6, 9, 12, 4, 7, 10, 13, 0, 1, 14, 15]

N_PARENT_FULL = 65536
N_CORES = 8
STRIDE = 2
NP = N_PARENT_FULL // N_CORES  # parents per core
BLK = 128

# float32r: fp32 with the mantissa truncated to 11 bits (TF32-like) but fp32
# PSUM accumulation; single-pass matmul (half the PE row-cost of fp32, one
# LDWEIGHTS instead of two). Input rounding error ~2.4e-4 relative.
FP32R_MM = True


def build_program(np_parents=NP):
    import concourse.mybir as mybir
    import concourse.tile as tile
    from concourse import bacc
    from concourse.masks import make_identity

    f32 = mybir.dt.float32
    xdt = mybir.dt.float32r if FP32R_MM else f32
    nch = np_parents * STRIDE
    nblk = np_parents // BLK
    assert np_parents % BLK == 0

    nc = bacc.Bacc("TRN2", target_bir_lowering=False, debug=False)

    x_mv = nc.dram_tensor("x_mv", [np_parents, 512], f32, kind="ExternalInput").ap()
    x_s = nc.dram_tensor("x_s", [np_parents, 64], f32, kind="ExternalInput").ap()
    skip_mv = nc.dram_tensor("skip_mv", [nch, 512], f32, kind="ExternalInput").ap()
    skip_s = nc.dram_tensor("skip_s", [nch, 64], f32, kind="ExternalInput").ap()

    # single packed weight tensor -> one DMA, one wait source for the matmuls
    wcat = nc.dram_tensor("wcat", [128, 864], f32, kind="ExternalInput").ap()

    out_mv = nc.dram_tensor("out_mv", [nch, 512], f32, kind="ExternalOutput").ap()
    out_s = nc.dram_tensor("out_s", [nch, 64], f32, kind="ExternalOutput").ap()

    with tile.TileContext(nc) as tc:
        with (
            tc.tile_pool(name="const", bufs=1) as cpool,
            tc.tile_pool(name="io", bufs=4) as iopool,
            tc.tile_pool(name="xt", bufs=3) as xtpool,
            tc.tile_pool(name="ps", bufs=2, space="PSUM") as pspool,
        ):
            ident = cpool.tile([128, 128], f32)
            make_identity(nc, ident[:])
            ones = cpool.tile([1, 128], f32)
            nc.gpsimd.memset(ones[:], 1.0)
            wt = cpool.tile([128, 864], f32)
            nc.sync.dma_start(wt[:], wcat)
            # rounded copies for the fp32r matmul path (ACT rounds on copy)
            wtr = cpool.tile([128, 864], xdt)
            nc.scalar.copy(wtr[:], wt[:])
            onesr = cpool.tile([1, 128], xdt)
            nc.scalar.copy(onesr[:], ones[:])

            # PE matmuls tolerate few semaphore waits; funnel PE deps through
            # ACT. These dummies absorb the one-time gpsimd (identity/ones)
            # and weight-DMA waits into PE's observed vector clock; the ACT
            # reads keep the psum slot releases on the ACT semaphore.
            d1 = pspool.tile([128, 512], f32, tag="ptA")
            nc.tensor.transpose(d1[:, 0:128], ident[:], ident[:])
            d2 = pspool.tile([128, 512], f32, tag="ptA")
            nc.tensor.transpose(d2[:, 0:128], wt[:, 0:128], ident[:])
            scratch = cpool.tile([128, 128], f32)
            nc.scalar.copy(scratch[:], d1[:, 0:128])
            nc.scalar.copy(scratch[:], d2[:, 0:128])

            # 2-block superblocks halve the DMA count (SWDGE fixed cost is
            # ~2us per transfer). DMA channel split: sync ring = x/x_s/skip_s
            # loads + out_s stores; SWDGE = skip_mv loads; ACT ring = out_mv.
            for sb in range(nblk // 2):
                p0 = sb * 2 * BLK
                c0 = p0 * STRIDE
                xc2 = iopool.tile([BLK, 1024], f32, tag="xc")
                nc.sync.dma_start(
                    xc2[:].rearrange("p (g f) -> p g f", g=2),
                    x_mv[p0 : p0 + 2 * BLK, :].rearrange("(g p) f -> p g f", g=2),
                )
                xcs2 = iopool.tile([BLK, 128], f32, tag="xcs")
                nc.sync.dma_start(
                    xcs2[:].rearrange("p (g s) -> p g s", g=2),
                    x_s[p0 : p0 + 2 * BLK, :].rearrange("(g p) s -> p g s", g=2),
                )
                skm2 = iopool.tile([BLK, 2048], f32, tag="skm")
                nc.gpsimd.dma_start(
                    skm2[:].rearrange("p (g a f) -> p g a f", g=2, a=2),
                    skip_mv[c0 : c0 + 4 * BLK, :].rearrange(
                        "(g p a) f -> p g a f", g=2, a=2
                    ),
                )
                sks2 = iopool.tile([BLK, 256], f32, tag="sks")
                nc.sync.dma_start(
                    sks2[:].rearrange("p (g a s) -> p g a s", g=2, a=2),
                    skip_s[c0 : c0 + 4 * BLK, :].rearrange(
                        "(g p a) s -> p g a s", g=2, a=2
                    ),
                )
                om2 = iopool.tile([BLK, 2048], f32, tag="om")
                os2 = iopool.tile([BLK, 256], f32, tag="os")

                for g in range(2):
                    xcg = xc2[:, g * 512 : (g + 1) * 512]
                    # --- GPSIMD: permute mv features (i,k) -> pair-order (q,i)
                    # so x^T comes out with wedge pairs co-located per K-tile
                    xcp = iopool.tile([BLK, 512], f32, tag="xcp")
                    x3 = xcg.rearrange("p (i k) -> p k i", k=16)  # [p, 16, 32]
                    xcp3 = xcp[:].rearrange("p (q i) -> p q i", i=32)
                    # positions 0:12 <- blades 2+t+3b in one lattice copy
                    nc.gpsimd.tensor_copy(
                        xcp3[:, 0:12, :].rearrange("p (t b) i -> p t b i", b=4),
                        x3[:, 2:14, :].rearrange("p (b t) i -> p t b i", t=3),
                    )
                    # tile 3: pairs (0,1) and (14,15)
                    nc.gpsimd.tensor_copy(xcp3[:, 12:14, :], x3[:, 0:2, :])
                    nc.gpsimd.tensor_copy(xcp3[:, 14:16, :], x3[:, 14:16, :])

                    ptA = pspool.tile([128, 512], f32, tag="ptA")
                    for t in range(4):
                        nc.tensor.transpose(
                            ptA[:, t * 128 : (t + 1) * 128],
                            xcp[:, t * 128 : (t + 1) * 128],
                            ident[:],
                        )
                    ptB = pspool.tile([64, 128], f32, tag="ptB")
                    nc.tensor.transpose(ptB[:], xcs2[:, g * 64 : (g + 1) * 64], ident[:])

                    xt = xtpool.tile([128, 512], xdt, tag="xt")
                    nc.vector.tensor_copy(xt[:], ptA[:])
                    xts = xtpool.tile([64, 128], xdt, tag="xts")
                    nc.vector.tensor_copy(xts[:], ptB[:])

                    # --- matmuls (psum col = q*32+o; K-tile t -> cols of t) ---
                    poA = pspool.tile([128, 512], f32, tag="poA")
                    poB = pspool.tile([128, 64], f32, tag="poB")
                    mm = nc.tensor.matmul
                    for t in range(4):
                        cs = slice(128 * t, 128 * (t + 1))
                        mm(poA[:, cs], xt[:, cs], wtr[:, cs], start=(t == 0), stop=False)
                    # blade 0 sits at position 12 (tile 3)
                    mm(poA[:, 384:416], xts[:], wtr[0:64, 704:736], start=False, stop=True)
                    mm(poB[:], xt[:, 384:512], wtr[:, 640:704], start=True, stop=False)
                    mm(poB[:], xts[:], wtr[0:64, 736:800], start=False, stop=False)
                    mm(poB[:], onesr[:], wtr[0:1, 800:864], start=False, stop=True)

                    # --- ACT drains PSUM -> SBUF, undoing the permutation ---
                    oA = xtpool.tile([BLK, 512], f32, tag="oA")
                    oA3 = oA[:].rearrange("p (o j) -> p o j", j=16)
                    poAq = poA[:].rearrange("p (q o) -> p q o", o=32)
                    nc.scalar.copy(
                        oA3[:, :, 2:14].rearrange("p o (b t) -> p o t b", t=3),
                        poA[:, 0:384].rearrange("p (t b o) -> p o t b", b=4, o=32),
                    )
                    nc.scalar.copy(oA3[:, :, 0:2], poAq[:, 12:14, :].transpose([0, 2, 1]))
                    nc.scalar.copy(oA3[:, :, 14:16], poAq[:, 14:16, :].transpose([0, 2, 1]))

                    # --- skip add (DVE reads poB straight from PSUM) ---
                    o0 = g * 1024
                    nc.vector.tensor_add(om2[:, o0 : o0 + 512], oA[:], skm2[:, o0 : o0 + 512])
                    nc.vector.tensor_add(
                        om2[:, o0 + 512 : o0 + 1024], oA[:], skm2[:, o0 + 512 : o0 + 1024]
                    )
                    s0 = g * 128
                    nc.vector.tensor_add(os2[:, s0 : s0 + 64], poB[:], sks2[:, s0 : s0 + 64])
                    nc.vector.tensor_add(
                        os2[:, s0 + 64 : s0 + 128], poB[:], sks2[:, s0 + 64 : s0 + 128]
                    )

                nc.scalar.dma_start(
                    out_mv[c0 : c0 + 4 * BLK, :].rearrange(
                        "(g p a) f -> p g a f", g=2, a=2
                    ),
                    om2[:].rearrange("p (g a f) -> p g a f", g=2, a=2),
                )
                nc.sync.dma_start(
                    out_s[c0 : c0 + 4 * BLK, :].rearrange(
                        "(g p a) s -> p g a s", g=2, a=2
                    ),
                    os2[:].rearrange("p (g a s) -> p g a s", g=2, a=2),
                )
    nc.compile()
    return nc


def pack_weights(w_mv, w_s2mv, w_mv2s, w_s2s, b_s):
    """Build the combined [577, 576] weight, then permute rows/cols into the
    pair-ordered layout and concatenate all matmul blocks into one tensor."""
    w_mv = np.asarray(w_mv, np.float32)
    W = np.zeros((577, 576), np.float32)
    for j in range(16):
        W[j * 32 : (j + 1) * 32, j:512:16] = w_mv[:, :, _GRADES[j]].T
    for src, dst in _E0_MAP.items():
        W[src * 32 : (src + 1) * 32, dst:512:16] += w_mv[:, :, 5 + _GRADES[src]].T
    W[512:576, 0:512:16] = np.asarray(w_s2mv, np.float32).T
    W[0:32, 512:576] = np.asarray(w_mv2s, np.float32).T
    W[512:576, 512:576] += np.asarray(w_s2s, np.float32).T
    W[576, 512:576] = np.asarray(b_s, np.float32)

    # permute rows and mv-cols into pair order: position q <-> blade SIGMA[q]
    row_perm = [_SIGMA[q] * 32 + i for q in range(16) for i in range(32)]
    col_perm = [o * 16 + _SIGMA[q] for q in range(16) for o in range(32)]
    Wmv = W[:, :512][row_perm + list(range(512, 577))][:, col_perm]
    Ws = W[:, 512:576]

    wcat = np.zeros((128, 864), np.float32)
    for t in range(4):
        wcat[:, 128 * t : 128 * (t + 1)] = Wmv[128 * t : 128 * (t + 1),
                                               128 * t : 128 * (t + 1)]
    wcat[:, 640:704] = Ws[row_perm[384:512]]      # out_s from tile-3 rows (blade 0)
    wcat[0:64, 704:736] = Wmv[512:576, 384:416]   # s2mv -> blade 0 (position 12)
    wcat[0:64, 736:800] = Ws[512:576]             # s2s
    wcat[0, 800:864] = Ws[576]                    # bias via ones row
    return {"wcat": wcat}


_cached_nc = None
TRACE = False  # set True (e.g. from test.py) to capture an NTFF profile
LAST_RESULTS = None  # BassKernelResults of the most recent kernel() call


def kernel(x_mv, x_s, skip_mv, skip_s, w_mv, w_s2mv, w_mv2s, w_s2s, b_s, stride):
    global _cached_nc, LAST_RESULTS
    from concourse.bass_utils import run_bass_kernel_spmd

    assert int(stride) == STRIDE
    x_mv = np.ascontiguousarray(np.asarray(x_mv, np.float32).reshape(N_PARENT_FULL, 512))
    x_s = np.ascontiguousarray(np.asarray(x_s, np.float32))
    n_child = N_PARENT_FULL * STRIDE
    skip_mv_f = np.ascontiguousarray(np.asarray(skip_mv, np.float32).reshape(n_child, 512))
    skip_s = np.ascontiguousarray(np.asarray(skip_s, np.float32))

    packed = pack_weights(w_mv, w_s2mv, w_mv2s, w_s2s, b_s)

    if _cached_nc is None:
        _cached_nc = build_program(NP)
    nc = _cached_nc

    nch = NP * STRIDE
    in_maps = []
    for c in range(N_CORES):
        m = {
            "x_mv": x_mv[c * NP : (c + 1) * NP],
            "x_s": x_s[c * NP : (c + 1) * NP],
            "skip_mv": skip_mv_f[c * nch : (c + 1) * nch],
            "skip_s": skip_s[c * nch : (c + 1) * nch],
        }
        m.update(packed)
        in_maps.append(m)

    res = run_bass_kernel_spmd(nc, in_maps, core_ids=list(range(N_CORES)), trace=TRACE)
    LAST_RESULTS = res
    out_mv = np.concatenate([r["out_mv"] for r in res.results], axis=0)
    out_s = np.concatenate([r["out_s"] for r in res.results], axis=0)
    return out_mv.reshape(n_child, 32, 16), out_s


# revision 32
# speedup vs baseline: 1.7390x; 1.0219x over previous
"""Trainium2 Bass kernel for EquivariantBallUnpooling.

Math (per parent node n):
  out[n, 0:576] = x_comb[n, 0:577] @ W_combined
where x_comb = [x_mv flattened (i,k), x_s, 1.0] and W_combined is built host-side
from the small EquiLinear weights and the 9-element Pin(3,0,1) basis. The basis
is block-sparse: output blade j receives only input blades {j, e0src(j)}.

Children = repeat_interleave(parents, 2), so all compute happens on parents; a
parent's result is added to both of its children's skip rows on-chip.

Blade tiling: the e0-wedge map pairs blades (0,1),(2,5),(3,6),(4,7),(8,11),
(9,12),(10,13),(14,15). x^T rows are permuted so each pair lives in one
128-row K-tile -> each K-tile feeds exactly its own 128 output columns:
4 dense [128x128] matmuls, no overlap, plus 3 small ones for the scalar head.

Per-core structure (pure data parallel over parents, 8 cores):
  per 128-parent block:
   - contiguous DMA loads (x 256KB, x_s 32KB, skip pair-tiles 512/64KB)
   - ACT permutes features to pair-order, PE transposes to x^T (PSUM->ACT->SBUF)
   - 8 matmuls accumulate out[128 parents, 576] in PSUM
   - ACT drains PSUM->SBUF undoing the pair permutation; DVE adds skip;
     contiguous stores (children of a parent share its partition)
"""

import numpy as np

_GRADES = np.array([0, 1, 1, 1, 1, 2, 2, 2, 2, 2, 2, 3, 3, 3, 3, 4])
_E0_MAP = {0: 1, 2: 5, 3: 6, 4: 7, 8: 11, 9: 12, 10: 13, 14: 15}

# x^T position q -> blade. Tiles (4 positions each) co-locate the e0-wedge
# pairs: T0={2,5,8,11} T1={3,6,9,12} T2={4,7,10,13} T3={0,1,14,15}. Positions
# 0..11 form the uniform lattice blade = 2 + t + 3b (q = 4t + b), so the
# permute/unpermute copies for 3 of 4 tiles collapse into one rearrange.
_SIGMA = [2, 5, 8, 11, 3, 6, 9, 12, 4, 7, 10, 13, 0, 1, 14, 15]

N_PARENT_FULL = 65536
N_CORES = 8
STRIDE = 2
NP = N_PARENT_FULL // N_CORES  # parents per core
BLK = 128

# float32r: fp32 with the mantissa truncated to 11 bits (TF32-like) but fp32
# PSUM accumulation; single-pass matmul (half the PE row-cost of fp32, one
# LDWEIGHTS instead of two). Input rounding error ~2.4e-4 relative.
FP32R_MM = True


def build_program(np_parents=NP):
    import concourse.mybir as mybir
    import concourse.tile as tile
    from concourse import bacc
    from concourse.masks import make_identity

    f32 = mybir.dt.float32
    xdt = mybir.dt.float32r if FP32R_MM else f32
    nch = np_parents * STRIDE
    nblk = np_parents // BLK
    assert np_parents % BLK == 0

    nc = bacc.Bacc("TRN2", target_bir_lowering=False, debug=False)

    x_mv = nc.dram_tensor("x_mv", [np_parents, 512], f32, kind="ExternalInput").ap()
    x_s = nc.dram_tensor("x_s", [np_parents, 64], f32, kind="ExternalInput").ap()
    skip_mv = nc.dram_tensor("skip_mv", [nch, 512], f32, kind="ExternalInput").ap()
    skip_s = nc.dram_tensor("skip_s", [nch, 64], f32, kind="ExternalInput").ap()

    # single packed weight tensor -> one DMA, one wait source for the matmuls
    wcat = nc.dram_tensor("wcat", [128, 864], f32, kind="ExternalInput").ap()

    out_mv = nc.dram_tensor("out_mv", [nch, 512], f32, kind="ExternalOutput").ap()
    out_s = nc.dram_tensor("out_s", [nch, 64], f32, kind="ExternalOutput").ap()

    with tile.TileContext(nc) as tc:
        with (
            tc.tile_pool(name="const", bufs=1) as cpool,
            tc.tile_pool(name="io", bufs=4) as iopool,
            tc.tile_pool(name="xt", bufs=3) as xtpool,
            tc.tile_pool(name="ps", bufs=2, space="PSUM") as pspool,
        ):
            ident = cpool.tile([128, 128], f32)
            make_identity(nc, ident[:])
            identr = cpool.tile([128, 128], xdt)
            nc.scalar.copy(identr[:], ident[:])
            ones = cpool.tile([1, 128], f32)
            nc.gpsimd.memset(ones[:], 1.0)
            wt = cpool.tile([128, 864], f32)
            nc.sync.dma_start(wt[:], wcat)
            # rounded copies for the fp32r matmul path (ACT rounds on copy)
            wtr = cpool.tile([128, 864], xdt)
            nc.scalar.copy(wtr[:], wt[:])
            onesr = cpool.tile([1, 128], xdt)
            nc.scalar.copy(onesr[:], ones[:])

            # PE matmuls tolerate few semaphore waits; funnel PE deps through
            # ACT. These dummies absorb the one-time gpsimd (identity/ones)
            # and weight-DMA waits into PE's observed vector clock; the ACT
            # reads keep the psum slot releases on the ACT semaphore.
            d1 = pspool.tile([128, 512], f32, tag="ptA")
            nc.tensor.transpose(d1[:, 0:128], ident[:], ident[:])
            d2 = pspool.tile([128, 512], f32, tag="ptA")
            nc.tensor.transpose(d2[:, 0:128], wt[:, 0:128], ident[:])
            scratch = cpool.tile([128, 128], f32)
            nc.scalar.copy(scratch[:], d1[:, 0:128])
            nc.scalar.copy(scratch[:], d2[:, 0:128])

            # 2-block superblocks halve the DMA count (SWDGE fixed cost is
            # ~2us per transfer). DMA channel split: sync ring = x/x_s/skip_s
            # loads + out_s stores; SWDGE = skip_mv loads; ACT ring = out_mv.
            for sb in range(nblk // 2):
                p0 = sb * 2 * BLK
                c0 = p0 * STRIDE
                xc2 = iopool.tile([BLK, 1024], f32, tag="xc")
                nc.sync.dma_start(
                    xc2[:].rearrange("p (g f) -> p g f", g=2),
                    x_mv[p0 : p0 + 2 * BLK, :].rearrange("(g p) f -> p g f", g=2),
                )
                xcs2 = iopool.tile([BLK, 128], f32, tag="xcs")
                nc.sync.dma_start(
                    xcs2[:].rearrange("p (g s) -> p g s", g=2),
                    x_s[p0 : p0 + 2 * BLK, :].rearrange("(g p) s -> p g s", g=2),
                )
                skm2 = iopool.tile([BLK, 2048], f32, tag="skm")
                nc.gpsimd.dma_start(
                    skm2[:].rearrange("p (g a f) -> p g a f", g=2, a=2),
                    skip_mv[c0 : c0 + 4 * BLK, :].rearrange(
                        "(g p a) f -> p g a f", g=2, a=2
                    ),
                )
                sks2 = iopool.tile([BLK, 256], f32, tag="sks")
                nc.sync.dma_start(
                    sks2[:].rearrange("p (g a s) -> p g a s", g=2, a=2),
                    skip_s[c0 : c0 + 4 * BLK, :].rearrange(
                        "(g p a) s -> p g a s", g=2, a=2
                    ),
                )
                om2 = iopool.tile([BLK, 2048], f32, tag="om")
                os2 = iopool.tile([BLK, 256], f32, tag="os")

                for g in range(2):
                    xcg = xc2[:, g * 512 : (g + 1) * 512]
                    # --- DVE: permute mv features (i,k) -> pair-order (q,i),
                    # rounding to f32r, so x^T has wedge pairs per K-tile and
                    # the PE transposes run in single-pass f32r
                    xcp = iopool.tile([BLK, 576], xdt, tag="xcp")
                    x3 = xcg.rearrange("p (i k) -> p k i", k=16)  # [p, 16, 32]
                    xcp3 = xcp[:, 0:512].rearrange("p (q i) -> p q i", i=32)
                    # positions 0:12 <- blades 2+t+3b in one lattice copy
                    nc.vector.tensor_copy(
                        xcp3[:, 0:12, :].rearrange("p (t b) i -> p t b i", b=4),
                        x3[:, 2:14, :].rearrange("p (b t) i -> p t b i", t=3),
                    )
                    # tile 3: pairs (0,1) and (14,15)
                    nc.vector.tensor_copy(xcp3[:, 12:14, :], x3[:, 0:2, :])
                    nc.vector.tensor_copy(xcp3[:, 14:16, :], x3[:, 14:16, :])
                    nc.vector.tensor_copy(xcp[:, 512:576], xcs2[:, g * 64 : (g + 1) * 64])

                    ptA = pspool.tile([128, 512], xdt, tag="ptA")
                    for t in range(4):
                        nc.tensor.transpose(
                            ptA[:, t * 128 : (t + 1) * 128],
                            xcp[:, t * 128 : (t + 1) * 128],
                            identr[:],
                        )
                    ptB = pspool.tile([64, 128], xdt, tag="ptB")
                    nc.tensor.transpose(ptB[:], xcp[:, 512:576], identr[:])

                    xt = xtpool.tile([128, 512], xdt, tag="xt")
                    nc.vector.tensor_copy(xt[:], ptA[:])
                    xts = xtpool.tile([64, 128], xdt, tag="xts")
                    nc.vector.tensor_copy(xts[:], ptB[:])

                    # --- matmuls (psum col = q*32+o; K-tile t -> cols of t) ---
                    poA = pspool.tile([128, 512], f32, tag="poA")
                    poB = pspool.tile([128, 64], f32, tag="poB")
                    mm = nc.tensor.matmul
                    for t in range(4):
                        cs = slice(128 * t, 128 * (t + 1))
                        mm(poA[:, cs], xt[:, cs], wtr[:, cs], start=(t == 0), stop=False)
                    # blade 0 sits at position 12 (tile 3)
                    mm(poA[:, 384:416], xts[:], wtr[0:64, 704:736], start=False, stop=True)
                    mm(poB[:], xt[:, 384:512], wtr[:, 640:704], start=True, stop=False)
                    mm(poB[:], xts[:], wtr[0:64, 736:800], start=False, stop=False)
                    mm(poB[:], onesr[:], wtr[0:1, 800:864], start=False, stop=True)

                    # --- ACT drains PSUM -> SBUF, undoing the permutation ---
                    oA = xtpool.tile([BLK, 512], f32, tag="oA")
                    oA3 = oA[:].rearrange("p (o j) -> p o j", j=16)
                    poAq = poA[:].rearrange("p (q o) -> p q o", o=32)
                    nc.scalar.copy(
                        oA3[:, :, 2:14].rearrange("p o (b t) -> p o t b", t=3),
                        poA[:, 0:384].rearrange("p (t b o) -> p o t b", b=4, o=32),
                    )
                    nc.scalar.copy(oA3[:, :, 0:2], poAq[:, 12:14, :].transpose([0, 2, 1]))
                    nc.scalar.copy(oA3[:, :, 14:16], poAq[:, 14:16, :].transpose([0, 2, 1]))

                    # --- skip add (DVE reads poB straight from PSUM) ---
                    o0 = g * 1024
                    nc.gpsimd.tensor_add(om2[:, o0 : o0 + 512], oA[:], skm2[:, o0 : o0 + 512])
                    nc.gpsimd.tensor_add(
                        om2[:, o0 + 512 : o0 + 1024], oA[:], skm2[:, o0 + 512 : o0 + 1024]
                    )
                    s0 = g * 128
                    nc.vector.tensor_add(os2[:, s0 : s0 + 64], poB[:], sks2[:, s0 : s0 + 64])
                    nc.vector.tensor_add(
                        os2[:, s0 + 64 : s0 + 128], poB[:], sks2[:, s0 + 64 : s0 + 128]
                    )

                nc.scalar.dma_start(
                    out_mv[c0 : c0 + 4 * BLK, :].rearrange(
                        "(g p a) f -> p g a f", g=2, a=2
                    ),
                    om2[:].rearrange("p (g a f) -> p g a f", g=2, a=2),
                )
                nc.sync.dma_start(
                    out_s[c0 : c0 + 4 * BLK, :].rearrange(
                        "(g p a) s -> p g a s", g=2, a=2
                    ),
                    os2[:].rearrange("p (g a s) -> p g a s", g=2, a=2),
                )
    nc.compile()
    return nc


def pack_weights(w_mv, w_s2mv, w_mv2s, w_s2s, b_s):
    """Build the combined [577, 576] weight, then permute rows/cols into the
    pair-ordered layout and concatenate all matmul blocks into one tensor."""
    w_mv = np.asarray(w_mv, np.float32)
    W = np.zeros((577, 576), np.float32)
    for j in range(16):
        W[j * 32 : (j + 1) * 32, j:512:16] = w_mv[:, :, _GRADES[j]].T
    for src, dst in _E0_MAP.items():
        W[src * 32 : (src + 1) * 32, dst:512:16] += w_mv[:, :, 5 + _GRADES[src]].T
    W[512:576, 0:512:16] = np.asarray(w_s2mv, np.float32).T
    W[0:32, 512:576] = np.asarray(w_mv2s, np.float32).T
    W[512:576, 512:576] += np.asarray(w_s2s, np.float32).T
    W[576, 512:576] = np.asarray(b_s, np.float32)

    # permute rows and mv-cols into pair order: position q <-> blade SIGMA[q]
    row_perm = [_SIGMA[q] * 32 + i for q in range(16) for i in range(32)]
    col_perm = [o * 16 + _SIGMA[q] for q in range(16) for o in range(32)]
    Wmv = W[:, :512][row_perm + list(range(512, 577))][:, col_perm]
    Ws = W[:, 512:576]

    wcat = np.zeros((128, 864), np.float32)
    for t in range(4):
        wcat[:, 128 * t : 128 * (t + 1)] = Wmv[128 * t : 128 * (t + 1),
                                               128 * t : 128 * (t + 1)]
    wcat[:, 640:704] = Ws[row_perm[384:512]]      # out_s from tile-3 rows (blade 0)
    wcat[0:64, 704:736] = Wmv[512:576, 384:416]   # s2mv -> blade 0 (position 12)
    wcat[0:64, 736:800] = Ws[512:576]             # s2s
    wcat[0, 800:864] = Ws[576]                    # bias via ones row
    return {"wcat": wcat}


_cached_nc = None
TRACE = False  # set True (e.g. from test.py) to capture an NTFF profile
LAST_RESULTS = None  # BassKernelResults of the most recent kernel() call


def kernel(x_mv, x_s, skip_mv, skip_s, w_mv, w_s2mv, w_mv2s, w_s2s, b_s, stride):
    global _cached_nc, LAST_RESULTS
    from concourse.bass_utils import run_bass_kernel_spmd

    assert int(stride) == STRIDE
    x_mv = np.ascontiguousarray(np.asarray(x_mv, np.float32).reshape(N_PARENT_FULL, 512))
    x_s = np.ascontiguousarray(np.asarray(x_s, np.float32))
    n_child = N_PARENT_FULL * STRIDE
    skip_mv_f = np.ascontiguousarray(np.asarray(skip_mv, np.float32).reshape(n_child, 512))
    skip_s = np.ascontiguousarray(np.asarray(skip_s, np.float32))

    packed = pack_weights(w_mv, w_s2mv, w_mv2s, w_s2s, b_s)

    if _cached_nc is None:
        _cached_nc = build_program(NP)
    nc = _cached_nc

    nch = NP * STRIDE
    in_maps = []
    for c in range(N_CORES):
        m = {
            "x_mv": x_mv[c * NP : (c + 1) * NP],
            "x_s": x_s[c * NP : (c + 1) * NP],
            "skip_mv": skip_mv_f[c * nch : (c + 1) * nch],
            "skip_s": skip_s[c * nch : (c + 1) * nch],
        }
        m.update(packed)
        in_maps.append(m)

    res = run_bass_kernel_spmd(nc, in_maps, core_ids=list(range(N_CORES)), trace=TRACE)
    LAST_RESULTS = res
    out_mv = np.concatenate([r["out_mv"] for r in res.results], axis=0)
    out_s = np.concatenate([r["out_s"] for r in res.results], axis=0)
    return out_mv.reshape(n_child, 32, 16), out_s


# revision 34
# speedup vs baseline: 1.7924x; 1.0307x over previous
"""Trainium2 Bass kernel for EquivariantBallUnpooling.

Math (per parent node n):
  out[n, 0:576] = x_comb[n, 0:577] @ W_combined
where x_comb = [x_mv flattened (i,k), x_s, 1.0] and W_combined is built host-side
from the small EquiLinear weights and the 9-element Pin(3,0,1) basis. The basis
is block-sparse: output blade j receives only input blades {j, e0src(j)}.

Children = repeat_interleave(parents, 2), so all compute happens on parents; a
parent's result is added to both of its children's skip rows on-chip.

Blade tiling: the e0-wedge map pairs blades (0,1),(2,5),(3,6),(4,7),(8,11),
(9,12),(10,13),(14,15). x^T rows are permuted so each pair lives in one
128-row K-tile -> each K-tile feeds exactly its own 128 output columns:
4 dense [128x128] matmuls, no overlap, plus 3 small ones for the scalar head.

Per-core structure (pure data parallel over parents, 8 cores):
  per 128-parent block:
   - contiguous DMA loads (x 256KB, x_s 32KB, skip pair-tiles 512/64KB)
   - ACT permutes features to pair-order, PE transposes to x^T (PSUM->ACT->SBUF)
   - 8 matmuls accumulate out[128 parents, 576] in PSUM
   - ACT drains PSUM->SBUF undoing the pair permutation; DVE adds skip;
     contiguous stores (children of a parent share its partition)
"""

import numpy as np

_GRADES = np.array([0, 1, 1, 1, 1, 2, 2, 2, 2, 2, 2, 3, 3, 3, 3, 4])
_E0_MAP = {0: 1, 2: 5, 3: 6, 4: 7, 8: 11, 9: 12, 10: 13, 14: 15}

# x^T position q -> blade. Tiles (4 positions each) co-locate the e0-wedge
# pairs: T0={2,5,8,11} T1={3,6,9,12} T2={4,7,10,13} T3={0,1,14,15}. Positions
# 0..11 form the uniform lattice blade = 2 + t + 3b (q = 4t + b), so the
# permute/unpermute copies for 3 of 4 tiles collapse into one rearrange.
_SIGMA = [2, 5, 8, 11, 3, 6, 9, 12, 4, 7, 10, 13, 0, 1, 14, 15]

N_PARENT_FULL = 65536
N_CORES = 8
STRIDE = 2
NP = N_PARENT_FULL // N_CORES  # parents per core
BLK = 128

# float32r: fp32 with the mantissa truncated to 11 bits (TF32-like) but fp32
# PSUM accumulation; single-pass matmul (half the PE row-cost of fp32, one
# LDWEIGHTS instead of two). Input rounding error ~2.4e-4 relative.
FP32R_MM = True


def build_program(np_parents=NP):
    import concourse.mybir as mybir
    import concourse.tile as tile
    from concourse import bacc
    from concourse.masks import make_identity

    f32 = mybir.dt.float32
    xdt = mybir.dt.float32r if FP32R_MM else f32
    nch = np_parents * STRIDE
    nblk = np_parents // BLK
    assert np_parents % BLK == 0

    nc = bacc.Bacc("TRN2", target_bir_lowering=False, debug=False)

    x_mv = nc.dram_tensor("x_mv", [np_parents, 512], f32, kind="ExternalInput").ap()
    x_s = nc.dram_tensor("x_s", [np_parents, 64], f32, kind="ExternalInput").ap()
    skip_mv = nc.dram_tensor("skip_mv", [nch, 512], f32, kind="ExternalInput").ap()
    skip_s = nc.dram_tensor("skip_s", [nch, 64], f32, kind="ExternalInput").ap()

    # single packed weight tensor -> one DMA, one wait source for the matmuls
    wcat = nc.dram_tensor("wcat", [128, 864], f32, kind="ExternalInput").ap()

    out_mv = nc.dram_tensor("out_mv", [nch, 512], f32, kind="ExternalOutput").ap()
    out_s = nc.dram_tensor("out_s", [nch, 64], f32, kind="ExternalOutput").ap()

    with tile.TileContext(nc) as tc:
        with (
            tc.tile_pool(name="const", bufs=1) as cpool,
            tc.tile_pool(name="io", bufs=4) as iopool,
            tc.tile_pool(name="xt", bufs=3) as xtpool,
            tc.tile_pool(name="ps", bufs=2, space="PSUM") as pspool,
        ):
            ident = cpool.tile([128, 128], f32)
            make_identity(nc, ident[:])
            identr = cpool.tile([128, 128], xdt)
            nc.scalar.copy(identr[:], ident[:])
            ones = cpool.tile([1, 128], f32)
            nc.gpsimd.memset(ones[:], 1.0)
            wt = cpool.tile([128, 864], f32)
            nc.sync.dma_start(wt[:], wcat)
            # rounded copies for the fp32r matmul path (ACT rounds on copy)
            wtr = cpool.tile([128, 864], xdt)
            nc.scalar.copy(wtr[:], wt[:])
            onesr = cpool.tile([1, 128], xdt)
            nc.scalar.copy(onesr[:], ones[:])

            # PE matmuls tolerate few semaphore waits; funnel PE deps through
            # ACT. These dummies absorb the one-time gpsimd (identity/ones)
            # and weight-DMA waits into PE's observed vector clock; the ACT
            # reads keep the psum slot releases on the ACT semaphore.
            d1 = pspool.tile([128, 512], f32, tag="ptA")
            nc.tensor.transpose(d1[:, 0:128], ident[:], ident[:])
            d2 = pspool.tile([128, 512], f32, tag="ptA")
            nc.tensor.transpose(d2[:, 0:128], wt[:, 0:128], ident[:])
            scratch = cpool.tile([128, 128], f32)
            nc.scalar.copy(scratch[:], d1[:, 0:128])
            nc.scalar.copy(scratch[:], d2[:, 0:128])

            # 2-block superblocks halve the DMA count (SWDGE fixed cost is
            # ~2us per transfer). DMA channel split: sync ring = x/x_s/skip_s
            # loads + out_s stores; SWDGE = skip_mv loads; ACT ring = out_mv.
            for sb in range(nblk // 2):
                p0 = sb * 2 * BLK
                c0 = p0 * STRIDE
                xc2 = iopool.tile([BLK, 1024], f32, tag="xc")
                nc.sync.dma_start(
                    xc2[:].rearrange("p (g f) -> p g f", g=2),
                    x_mv[p0 : p0 + 2 * BLK, :].rearrange("(g p) f -> p g f", g=2),
                )
                xcs2 = iopool.tile([BLK, 128], f32, tag="xcs")
                nc.sync.dma_start(
                    xcs2[:].rearrange("p (g s) -> p g s", g=2),
                    x_s[p0 : p0 + 2 * BLK, :].rearrange("(g p) s -> p g s", g=2),
                )
                skm2 = iopool.tile([BLK, 2048], f32, tag="skm")
                nc.gpsimd.dma_start(
                    skm2[:].rearrange("p (g a f) -> p g a f", g=2, a=2),
                    skip_mv[c0 : c0 + 4 * BLK, :].rearrange(
                        "(g p a) f -> p g a f", g=2, a=2
                    ),
                )
                sks2 = iopool.tile([BLK, 256], f32, tag="sks")
                nc.sync.dma_start(
                    sks2[:].rearrange("p (g a s) -> p g a s", g=2, a=2),
                    skip_s[c0 : c0 + 4 * BLK, :].rearrange(
                        "(g p a) s -> p g a s", g=2, a=2
                    ),
                )
                om2 = iopool.tile([BLK, 2048], f32, tag="om")
                os2 = iopool.tile([BLK, 256], f32, tag="os")

                for g in range(2):
                    xcg = xc2[:, g * 512 : (g + 1) * 512]
                    # --- DVE: permute mv features (i,k) -> pair-order (q,i),
                    # rounding to f32r, so x^T has wedge pairs per K-tile and
                    # the PE transposes run in single-pass f32r
                    xcp = iopool.tile([BLK, 576], xdt, tag="xcp")
                    x3 = xcg.rearrange("p (i k) -> p k i", k=16)  # [p, 16, 32]
                    xcp3 = xcp[:, 0:512].rearrange("p (q i) -> p q i", i=32)
                    # positions 0:12 <- blades 2+t+3b in one lattice copy
                    nc.scalar.copy(
                        xcp3[:, 0:12, :].rearrange("p (t b) i -> p t b i", b=4),
                        x3[:, 2:14, :].rearrange("p (b t) i -> p t b i", t=3),
                    )
                    # tile 3: pairs (0,1) and (14,15)
                    nc.vector.tensor_copy(xcp3[:, 12:14, :], x3[:, 0:2, :])
                    nc.vector.tensor_copy(xcp3[:, 14:16, :], x3[:, 14:16, :])
                    nc.vector.tensor_copy(xcp[:, 512:576], xcs2[:, g * 64 : (g + 1) * 64])

                    ptA = pspool.tile([128, 512], xdt, tag="ptA")
                    for t in range(4):
                        nc.tensor.transpose(
                            ptA[:, t * 128 : (t + 1) * 128],
                            xcp[:, t * 128 : (t + 1) * 128],
                            identr[:],
                        )
                    ptB = pspool.tile([64, 128], xdt, tag="ptB")
                    nc.tensor.transpose(ptB[:], xcp[:, 512:576], identr[:])

                    xt = xtpool.tile([128, 512], xdt, tag="xt")
                    nc.vector.tensor_copy(xt[:], ptA[:])
                    xts = xtpool.tile([65, 128], xdt, tag="xts")
                    nc.vector.tensor_copy(xts[0:64, :], ptB[:])
                    nc.vector.tensor_copy(xts[64:65, :], onesr[:])

                    # --- matmuls (psum col = q*32+o; K-tile t -> cols of t) ---
                    poA = pspool.tile([128, 512], f32, tag="poA")
                    poB = pspool.tile([128, 64], f32, tag="poB")
                    mm = nc.tensor.matmul
                    for t in range(4):
                        cs = slice(128 * t, 128 * (t + 1))
                        mm(poA[:, cs], xt[:, cs], wtr[:, cs], start=(t == 0), stop=False)
                    # blade 0 sits at position 12 (tile 3); xts row 64 is
                    # the ones row carrying the bias through wtr row 64
                    mm(poA[:, 384:416], xts[:], wtr[0:65, 704:736], start=False, stop=True)
                    mm(poB[:], xt[:, 384:512], wtr[:, 640:704], start=True, stop=False)
                    mm(poB[:], xts[:], wtr[0:65, 736:800], start=False, stop=True)

                    # --- ACT drains PSUM -> SBUF, undoing the permutation
                    # (strided PSUM reads are ACT-safe; DVE hit a HW fault) ---
                    oA = xtpool.tile([BLK, 512], f32, tag="oA")
                    oA3 = oA[:].rearrange("p (o j) -> p o j", j=16)
                    poAq = poA[:].rearrange("p (q o) -> p q o", o=32)
                    nc.scalar.copy(
                        oA3[:, :, 2:14].rearrange("p o (b t) -> p o t b", t=3),
                        poA[:, 0:384].rearrange("p (t b o) -> p o t b", b=4, o=32),
                    )
                    nc.scalar.copy(oA3[:, :, 0:2], poAq[:, 12:14, :].transpose([0, 2, 1]))
                    nc.scalar.copy(oA3[:, :, 14:16], poAq[:, 14:16, :].transpose([0, 2, 1]))

                    # --- skip add ---
                    o0 = g * 1024
                    nc.gpsimd.tensor_add(om2[:, o0 : o0 + 512], oA[:], skm2[:, o0 : o0 + 512])
                    nc.vector.tensor_add(
                        om2[:, o0 + 512 : o0 + 1024], oA[:], skm2[:, o0 + 512 : o0 + 1024]
                    )
                    s0 = g * 128
                    nc.vector.tensor_add(os2[:, s0 : s0 + 64], poB[:], sks2[:, s0 : s0 + 64])
                    nc.vector.tensor_add(
                        os2[:, s0 + 64 : s0 + 128], poB[:], sks2[:, s0 + 64 : s0 + 128]
                    )

                nc.scalar.dma_start(
                    out_mv[c0 : c0 + 4 * BLK, :].rearrange(
                        "(g p a) f -> p g a f", g=2, a=2
                    ),
                    om2[:].rearrange("p (g a f) -> p g a f", g=2, a=2),
                )
                nc.sync.dma_start(
                    out_s[c0 : c0 + 4 * BLK, :].rearrange(
                        "(g p a) s -> p g a s", g=2, a=2
                    ),
                    os2[:].rearrange("p (g a s) -> p g a s", g=2, a=2),
                )
    nc.compile()
    return nc


def pack_weights(w_mv, w_s2mv, w_mv2s, w_s2s, b_s):
    """Build the combined [577, 576] weight, then permute rows/cols into the
    pair-ordered layout and concatenate all matmul blocks into one tensor."""
    w_mv = np.asarray(w_mv, np.float32)
    W = np.zeros((577, 576), np.float32)
    for j in range(16):
        W[j * 32 : (j + 1) * 32, j:512:16] = w_mv[:, :, _GRADES[j]].T
    for src, dst in _E0_MAP.items():
        W[src * 32 : (src + 1) * 32, dst:512:16] += w_mv[:, :, 5 + _GRADES[src]].T
    W[512:576, 0:512:16] = np.asarray(w_s2mv, np.float32).T
    W[0:32, 512:576] = np.asarray(w_mv2s, np.float32).T
    W[512:576, 512:576] += np.asarray(w_s2s, np.float32).T
    W[576, 512:576] = np.asarray(b_s, np.float32)

    # permute rows and mv-cols into pair order: position q <-> blade SIGMA[q]
    row_perm = [_SIGMA[q] * 32 + i for q in range(16) for i in range(32)]
    col_perm = [o * 16 + _SIGMA[q] for q in range(16) for o in range(32)]
    Wmv = W[:, :512][row_perm + list(range(512, 577))][:, col_perm]
    Ws = W[:, 512:576]

    wcat = np.zeros((128, 864), np.float32)
    for t in range(4):
        wcat[:, 128 * t : 128 * (t + 1)] = Wmv[128 * t : 128 * (t + 1),
                                               128 * t : 128 * (t + 1)]
    wcat[:, 640:704] = Ws[row_perm[384:512]]      # out_s from tile-3 rows (blade 0)
    wcat[0:64, 704:736] = Wmv[512:576, 384:416]   # s2mv -> blade 0 (position 12)
    wcat[0:64, 736:800] = Ws[512:576]             # s2s
    wcat[64, 736:800] = Ws[576]                   # bias via xts ones row
    return {"wcat": wcat}


_cached_nc = None
TRACE = False  # set True (e.g. from test.py) to capture an NTFF profile
LAST_RESULTS = None  # BassKernelResults of the most recent kernel() call


def kernel(x_mv, x_s, skip_mv, skip_s, w_mv, w_s2mv, w_mv2s, w_s2s, b_s, stride):
    global _cached_nc, LAST_RESULTS
    from concourse.bass_utils import run_bass_kernel_spmd

    assert int(stride) == STRIDE
    x_mv = np.ascontiguousarray(np.asarray(x_mv, np.float32).reshape(N_PARENT_FULL, 512))
    x_s = np.ascontiguousarray(np.asarray(x_s, np.float32))
    n_child = N_PARENT_FULL * STRIDE
    skip_mv_f = np.ascontiguousarray(np.asarray(skip_mv, np.float32).reshape(n_child, 512))
    skip_s = np.ascontiguousarray(np.asarray(skip_s, np.float32))

    packed = pack_weights(w_mv, w_s2mv, w_mv2s, w_s2s, b_s)

    if _cached_nc is None:
        _cached_nc = build_program(NP)
    nc = _cached_nc

    nch = NP * STRIDE
    in_maps = []
    for c in range(N_CORES):
        m = {
            "x_mv": x_mv[c * NP : (c + 1) * NP],
            "x_s": x_s[c * NP : (c + 1) * NP],
            "skip_mv": skip_mv_f[c * nch : (c + 1) * nch],
            "skip_s": skip_s[c * nch : (c + 1) * nch],
        }
        m.update(packed)
        in_maps.append(m)

    res = run_bass_kernel_spmd(nc, in_maps, core_ids=list(range(N_CORES)), trace=TRACE)
    LAST_RESULTS = res
    out_mv = np.concatenate([r["out_mv"] for r in res.results], axis=0)
    out_s = np.concatenate([r["out_s"] for r in res.results], axis=0)
    return out_mv.reshape(n_child, 32, 16), out_s
